# revision 5
# baseline (speedup 1.0000x reference)
"""ATR (twin-gate RNN) Trainium2 kernel.

  p = x @ W1.T + b1                       (batched GEMM over all T)
  h_t = sig(p_t+q_t)*p_t + sig(p_t-q_t)*q_t,  q_t = h_{t-1} @ W2.T + b2

Sharding: data-parallel over batch B=128 -> 16 per core across 8 NeuronCores,
zero cross-core communication. All tensors are kept on-device in a
[d-partition, batch-free] (transposed) layout; host prepares/unprepares.

Matmul operands are fp16 (measured ~5e-4 max rel err vs fp32 reference),
accumulation fp32 in PSUM.
"""

import contextlib

import numpy as np

import concourse.bass as bass
import concourse.mybir as mybir
import concourse.tile as tile
from concourse import bacc
from concourse.bass import ts
from concourse.bass_utils import run_bass_kernel_spmd

B, T, D = 128, 256, 1024
NCORES = 8
BL = B // NCORES          # 16 batches per core
P = 128                   # partitions
DT = D // P               # 8 d-tiles
HDT = DT // 2             # 4 (half)
TW = 32                   # t-window per p-GEMM chunk (TW*BL = 512 cols)
NCH = T // TW             # 8 chunks
F16 = mybir.dt.float16
F32 = mybir.dt.float32

_CACHE = {}


def _emit(nc, xT, w1, w2, b1, b2bc, h0, outT):
    tc = nc._tc
    with contextlib.ExitStack() as ctx:
        singles = ctx.enter_context(tc.tile_pool(name="singles", bufs=1))
        w1_sb = singles.tile([P, DT, D], F16)
        nc.sync.dma_start(out=w1_sb, in_=w1)
        w2_sb = singles.tile([P, DT, D], F16)
        nc.sync.dma_start(out=w2_sb, in_=w2)
        b1_sb = singles.tile([P, DT], F32)
        nc.sync.dma_start(out=b1_sb, in_=b1)
        b2_sb = singles.tile([P, DT, BL], F32)
        nc.sync.dma_start(out=b2_sb, in_=b2bc)
        h0_sb = singles.tile([P, DT, BL], F16)
        nc.sync.dma_start(out=h0_sb, in_=h0)
        p_sb = singles.tile([P, DT, T, BL], F16)

        xT_r = xT.rearrange("(a p) t b -> a p t b", p=P)

        # ---- stage 1: p = x @ W1.T + b1, stored transposed as p_sb[e, t, b]
        with (
            tc.tile_pool(name="xin", bufs=3) as xpool,
            tc.tile_pool(name="pps", bufs=4, space="PSUM") as ppsum,
        ):
            for n in range(NCH):
                xn = xpool.tile([P, DT, TW, BL], F16)
                for dt in range(DT):
                    nc.sync.dma_start(out=xn[:, dt], in_=xT_r[dt, :, ts(n, TW), :])
                for e in range(DT):
                    ps = ppsum.tile([P, TW * BL], F32)
                    for dt in range(DT):
                        nc.tensor.matmul(
                            ps,
                            lhsT=w1_sb[:, dt, ts(e, P)],
                            rhs=xn[:, dt].rearrange("p t b -> p (t b)"),
                            start=(dt == 0),
                            stop=(dt == DT - 1),
                        )
                    nc.scalar.activation(
                        out=p_sb[:, e, ts(n, TW), :].rearrange("p t b -> p (t b)"),
                        in_=ps,
                        func=mybir.ActivationFunctionType.Identity,
                        bias=b1_sb[:, e : e + 1],
                        scale=1.0,
                    )

        # ---- stage 2: sequential recurrence over T
        with contextlib.ExitStack() as rctx:
            qps = rctx.enter_context(tc.tile_pool(name="qps", bufs=2, space="PSUM"))
            hp = rctx.enter_context(tc.tile_pool(name="hp", bufs=3))
            gp = rctx.enter_context(tc.tile_pool(name="gp", bufs=3))

            hA = h0_sb[:, 0:HDT, :]
            hB = h0_sb[:, HDT:DT, :]
            SIG = mybir.ActivationFunctionType.Sigmoid
            for t in range(T):
                # PSUM accumulation-group `start` clears whole-bank
                # has_written flags, so groups must never interleave within a
                # bank. A-half (e 0..3) is split into two banks by d-half so
                # its q completes at MM 32 of 64 and its gate chain (which
                # feeds the next step's first matmuls) starts early; B-half
                # accumulates d 0..7 in one bank, finishing at MM 64.
                qA0 = qps.tile([P, HDT, BL], F32, tag="qa0", name=f"qA0_{t}")
                qA1 = qps.tile([P, HDT, BL], F32, tag="qa1", name=f"qA1_{t}")
                qB = qps.tile([P, HDT, BL], F32, tag="qb", name=f"qB_{t}")
                for e in range(HDT):
                    for dl in range(HDT):
                        nc.tensor.matmul(
                            qA0[:, e, :],
                            lhsT=w2_sb[:, dl, ts(e, P)],
                            rhs=hA[:, dl, :],
                            start=(dl == 0),
                            stop=(dl == HDT - 1),
                        )
                for e in range(HDT):
                    for dl in range(HDT):
                        nc.tensor.matmul(
                            qA1[:, e, :],
                            lhsT=w2_sb[:, HDT + dl, ts(e, P)],
                            rhs=hB[:, dl, :],
                            start=(dl == 0),
                            stop=(dl == HDT - 1),
                        )
                for e in range(HDT, DT):
                    for d in range(DT):
                        hsrc = hA if d < HDT else hB
                        nc.tensor.matmul(
                            qB[:, e - HDT, :],
                            lhsT=w2_sb[:, d, ts(e, P)],
                            rhs=hsrc[:, d % HDT, :],
                            start=(d == 0),
                            stop=(d == DT - 1),
                        )

                # --- A-half gates (critical path into next step) ---
                ptA = p_sb[:, 0:HDT, t, :]
                qb0A = gp.tile([P, HDT, BL], F32, tag="qb0A")
                nc.vector.tensor_add(qb0A, qA0, b2_sb[:, 0:HDT, :])
                qbA = gp.tile([P, HDT, BL], F16, tag="qbA")
                nc.vector.tensor_add(qbA, qb0A, qA1)
                sA = gp.tile([P, HDT, BL], F16, tag="sA")
                nc.vector.tensor_add(sA, ptA, qbA)
                dA = gp.tile([P, HDT, BL], F16, tag="dA")
                nc.vector.tensor_sub(dA, ptA, qbA)
                igA = gp.tile([P, HDT, BL], F16, tag="igA")
                nc.scalar.activation(out=igA, in_=sA, func=SIG)
                fgA = gp.tile([P, HDT, BL], F16, tag="fgA")
                nc.scalar.activation(out=fgA, in_=dA, func=SIG)
                t1A = gp.tile([P, HDT, BL], F16, tag="t1A")
                nc.vector.tensor_mul(t1A, igA, ptA)
                t2A = gp.tile([P, HDT, BL], F16, tag="t2A")
                nc.vector.tensor_mul(t2A, fgA, qbA)
                hnA = hp.tile([P, HDT, BL], F16, tag="hA")
                nc.vector.tensor_add(hnA, t1A, t2A)
                nc.sync.dma_start(out=outT[t][:, 0:HDT, :], in_=hnA)

                # --- B-half gates (slack: needed at MM 17 of next step) ---
                ptB = p_sb[:, HDT:DT, t, :]
                qbB = gp.tile([P, HDT, BL], F16, tag="qbB")
                nc.vector.tensor_add(qbB, qB, b2_sb[:, HDT:DT, :])
                sB = gp.tile([P, HDT, BL], F16, tag="sB")
                nc.vector.tensor_add(sB, ptB, qbB)
                dB = gp.tile([P, HDT, BL], F16, tag="dB")
                nc.vector.tensor_sub(dB, ptB, qbB)
                igB = gp.tile([P, HDT, BL], F16, tag="igB")
                nc.scalar.activation(out=igB, in_=sB, func=SIG)
                fgB = gp.tile([P, HDT, BL], F16, tag="fgB")
                nc.scalar.activation(out=fgB, in_=dB, func=SIG)
                t1B = gp.tile([P, HDT, BL], F16, tag="t1B")
                nc.vector.tensor_mul(t1B, igB, ptB)
                t2B = gp.tile([P, HDT, BL], F16, tag="t2B")
                nc.vector.tensor_mul(t2B, fgB, qbB)
                hnB = hp.tile([P, HDT, BL], F16, tag="hB")
                nc.vector.tensor_add(hnB, t1B, t2B)
                nc.sync.dma_start(out=outT[t][:, HDT:DT, :], in_=hnB)

                hA, hB = hnA, hnB


def build():
    if "nc" in _CACHE:
        return _CACHE["nc"]
    nc = bacc.Bacc("TRN2", target_bir_lowering=False, debug=False, num_devices=NCORES)
    xT = nc.dram_tensor("xT", [D, T, BL], F16, kind="ExternalInput").ap()
    w1 = nc.dram_tensor("w1", [P, DT, D], F16, kind="ExternalInput").ap()
    w2 = nc.dram_tensor("w2", [P, DT, D], F16, kind="ExternalInput").ap()
    b1 = nc.dram_tensor("b1", [P, DT], F32, kind="ExternalInput").ap()
    b2bc = nc.dram_tensor("b2bc", [P, DT, BL], F32, kind="ExternalInput").ap()
    h0 = nc.dram_tensor("h0", [P, DT, BL], F16, kind="ExternalInput").ap()
    outT = nc.dram_tensor("outT", [T, P, DT, BL], F16, kind="ExternalOutput").ap()
    with tile.TileContext(nc) as tc:
        nc._tc = tc
        _emit(nc, xT, w1, w2, b1, b2bc, h0, outT)
    nc.compile()
    _CACHE["nc"] = nc
    return nc


def make_in_maps(x, W1, b1, W2, b2, init_hx):
    x = np.asarray(x, dtype=np.float32)
    W1 = np.asarray(W1, dtype=np.float32)
    b1 = np.asarray(b1, dtype=np.float32)
    W2 = np.asarray(W2, dtype=np.float32)
    b2 = np.asarray(b2, dtype=np.float32)
    init_hx = np.asarray(init_hx, dtype=np.float32)

    w1s = np.ascontiguousarray(
        W1.T.reshape(DT, P, D).transpose(1, 0, 2)
    ).astype(np.float16)  # [din, dtile, e] = W1[e, d]
    w2s = np.ascontiguousarray(
        W2.T.reshape(DT, P, D).transpose(1, 0, 2)
    ).astype(np.float16)
    b1s = np.ascontiguousarray(b1.reshape(DT, P).T)  # [e_in, e_tile]
    b2bc = np.ascontiguousarray(
        np.broadcast_to(b2.reshape(DT, P).T[:, :, None], (P, DT, BL))
    )
    h0 = np.ascontiguousarray(
        np.broadcast_to(init_hx.reshape(DT, P).T[:, :, None], (P, DT, BL))
    ).astype(np.float16)

    in_maps = []
    for c in range(NCORES):
        xc = x[c * BL : (c + 1) * BL]  # [BL, T, D]
        xTc = np.ascontiguousarray(xc.transpose(2, 1, 0)).astype(np.float16)
        in_maps.append(
            {"xT": xTc, "w1": w1s, "w2": w2s, "b1": b1s, "b2bc": b2bc, "h0": h0}
        )
    return in_maps


def assemble(results):
    out = np.empty((B, T, D), dtype=np.float32)
    for c in range(NCORES):
        oT = results[c]["outT"]  # [T, P, DT, BL] f16
        out[c * BL : (c + 1) * BL] = (
            oT.transpose(3, 0, 2, 1).reshape(BL, T, D).astype(np.float32)
        )
    return out


def run(inputs, trace=False):
    nc = build()
    in_maps = make_in_maps(**inputs)
    res = run_bass_kernel_spmd(nc, in_maps, list(range(NCORES)), trace=trace)
    return assemble(res.results), res


def kernel(x, W1, b1, W2, b2, init_hx):
    out, _ = run(dict(x=x, W1=W1, b1=b1, W2=W2, b2=b2, init_hx=init_hx))
    return out


# revision 8
# speedup vs baseline: 1.1027x; 1.1027x over previous
"""ATR (twin-gate RNN) Trainium2 kernel.

  p = x @ W1.T + b1                       (batched GEMM over all T)
  h_t = sig(p_t+q_t)*p_t + sig(p_t-q_t)*q_t,  q_t = h_{t-1} @ W2.T + b2

Sharding: data-parallel over batch B=128 -> 16 per core across 8 NeuronCores,
zero cross-core communication. All tensors are kept on-device in a
[d-partition, batch-free] (transposed) layout; host prepares/unprepares.

Matmul operands are fp16 (measured ~5e-4 max rel err vs fp32 reference),
accumulation fp32 in PSUM.
"""

import contextlib

import numpy as np

import concourse.bass as bass
import concourse.mybir as mybir
import concourse.tile as tile
from concourse import bacc
from concourse.bass import ts
from concourse.bass_utils import run_bass_kernel_spmd
from concourse.tile import add_dep_helper


def _chain(insts, reason):
    """Force issue order within an engine queue (no extra semaphores).

    add_dep_helper(x, y) makes x wait on y, so the later op is the first arg.
    """
    for a, b in zip(insts, insts[1:]):
        add_dep_helper(b.ins, a.ins, sync=False, reason=reason)

B, T, D = 128, 256, 1024
NCORES = 8
BL = B // NCORES          # 16 batches per core
P = 128                   # partitions
DT = D // P               # 8 d-tiles
HDT = DT // 2             # 4 (half)
TW = 32                   # t-window per p-GEMM chunk (TW*BL = 512 cols)
NCH = T // TW             # 8 chunks
F16 = mybir.dt.float16
F32 = mybir.dt.float32

_CACHE = {}


def _emit(nc, xT, w1, w2, b1, b2bc, h0, outT):
    tc = nc._tc
    with contextlib.ExitStack() as ctx:
        singles = ctx.enter_context(tc.tile_pool(name="singles", bufs=1))
        w1_sb = singles.tile([P, DT, D], F16)
        nc.sync.dma_start(out=w1_sb, in_=w1)
        w2_sb = singles.tile([P, DT, D], F16)
        nc.sync.dma_start(out=w2_sb, in_=w2)
        b1_sb = singles.tile([P, DT], F32)
        nc.sync.dma_start(out=b1_sb, in_=b1)
        b2_sb = singles.tile([P, DT, BL], F32)
        nc.sync.dma_start(out=b2_sb, in_=b2bc)
        h0_sb = singles.tile([P, DT, BL], F16)
        nc.sync.dma_start(out=h0_sb, in_=h0)
        p_sb = singles.tile([P, DT, T, BL], F16)

        xT_r = xT.rearrange("(a p) t b -> a p t b", p=P)

        # ---- stage 1: p = x @ W1.T + b1, stored transposed as p_sb[e, t, b]
        with (
            tc.tile_pool(name="xin", bufs=3) as xpool,
            tc.tile_pool(name="pps", bufs=4, space="PSUM") as ppsum,
        ):
            for n in range(NCH):
                xn = xpool.tile([P, DT, TW, BL], F16)
                for dt in range(DT):
                    nc.sync.dma_start(out=xn[:, dt], in_=xT_r[dt, :, ts(n, TW), :])
                for e in range(DT):
                    ps = ppsum.tile([P, TW * BL], F32)
                    for dt in range(DT):
                        nc.tensor.matmul(
                            ps,
                            lhsT=w1_sb[:, dt, ts(e, P)],
                            rhs=xn[:, dt].rearrange("p t b -> p (t b)"),
                            start=(dt == 0),
                            stop=(dt == DT - 1),
                        )
                    nc.scalar.activation(
                        out=p_sb[:, e, ts(n, TW), :].rearrange("p t b -> p (t b)"),
                        in_=ps,
                        func=mybir.ActivationFunctionType.Identity,
                        bias=b1_sb[:, e : e + 1],
                        scale=1.0,
                    )

        # ---- stage 2: sequential recurrence over T
        with contextlib.ExitStack() as rctx:
            qps = rctx.enter_context(tc.tile_pool(name="qps", bufs=2, space="PSUM"))
            hp = rctx.enter_context(tc.tile_pool(name="hp", bufs=3))
            gp = rctx.enter_context(tc.tile_pool(name="gp", bufs=3))

            hA = h0_sb[:, 0:HDT, :]
            hB = h0_sb[:, HDT:DT, :]
            SIG = mybir.ActivationFunctionType.Sigmoid
            for t in range(T):
                # PSUM accumulation-group `start` clears whole-bank
                # has_written flags, so groups must never interleave within a
                # bank. A-half (e 0..3) is split into two banks by d-half so
                # its q completes at MM 32 of 64 and its gate chain (which
                # feeds the next step's first matmuls) starts early; B-half
                # accumulates d 0..7 in one bank, finishing at MM 64.
                qA0 = qps.tile([P, HDT, BL], F32, tag="qa0", name=f"qA0_{t}")
                qA1 = qps.tile([P, HDT, BL], F32, tag="qa1", name=f"qA1_{t}")
                qB = qps.tile([P, HDT, BL], F32, tag="qb", name=f"qB_{t}")
                for e in range(HDT):
                    for dl in range(HDT):
                        nc.tensor.matmul(
                            qA0[:, e, :],
                            lhsT=w2_sb[:, dl, ts(e, P)],
                            rhs=hA[:, dl, :],
                            start=(dl == 0),
                            stop=(dl == HDT - 1),
                        )
                for e in range(HDT):
                    for dl in range(HDT):
                        nc.tensor.matmul(
                            qA1[:, e, :],
                            lhsT=w2_sb[:, HDT + dl, ts(e, P)],
                            rhs=hB[:, dl, :],
                            start=(dl == 0),
                            stop=(dl == HDT - 1),
                        )
                for e in range(HDT, DT):
                    for d in range(DT):
                        hsrc = hA if d < HDT else hB
                        nc.tensor.matmul(
                            qB[:, e - HDT, :],
                            lhsT=w2_sb[:, d, ts(e, P)],
                            rhs=hsrc[:, d % HDT, :],
                            start=(d == 0),
                            stop=(d == DT - 1),
                        )

                # --- gates: A-half chain is the critical path into the next
                # step; B-half ops are interleaved to fill DVE idle slots
                # while ACT computes the A sigmoids.
                ptA = p_sb[:, 0:HDT, t, :]
                ptB = p_sb[:, HDT:DT, t, :]
                qb0A = gp.tile([P, HDT, BL], F32, tag="qb0A")
                i_qb0A = nc.vector.tensor_add(qb0A, qA0, b2_sb[:, 0:HDT, :])
                qbA = gp.tile([P, HDT, BL], F16, tag="qbA")
                i_qbA = nc.vector.tensor_add(qbA, qb0A, qA1)
                sA = gp.tile([P, HDT, BL], F16, tag="sA")
                i_sA = nc.vector.tensor_add(sA, ptA, qbA)
                dA = gp.tile([P, HDT, BL], F16, tag="dA")
                i_dA = nc.vector.tensor_sub(dA, ptA, qbA)
                igA = gp.tile([P, HDT, BL], F16, tag="igA")
                i_igA = nc.scalar.activation(out=igA, in_=sA, func=SIG)
                fgA = gp.tile([P, HDT, BL], F16, tag="fgA")
                i_fgA = nc.scalar.activation(out=fgA, in_=dA, func=SIG)
                qbB = gp.tile([P, HDT, BL], F16, tag="qbB")
                i_qbB = nc.vector.tensor_add(qbB, qB, b2_sb[:, HDT:DT, :])
                sB = gp.tile([P, HDT, BL], F16, tag="sB")
                i_sB = nc.vector.tensor_add(sB, ptB, qbB)
                dB = gp.tile([P, HDT, BL], F16, tag="dB")
                i_dB = nc.vector.tensor_sub(dB, ptB, qbB)
                igB = gp.tile([P, HDT, BL], F16, tag="igB")
                i_igB = nc.scalar.activation(out=igB, in_=sB, func=SIG)
                fgB = gp.tile([P, HDT, BL], F16, tag="fgB")
                i_fgB = nc.scalar.activation(out=fgB, in_=dB, func=SIG)
                t1A = gp.tile([P, HDT, BL], F16, tag="t1A")
                i_t1A = nc.vector.tensor_mul(t1A, igA, ptA)
                t2A = gp.tile([P, HDT, BL], F16, tag="t2A")
                i_t2A = nc.vector.tensor_mul(t2A, fgA, qbA)
                hnA = hp.tile([P, HDT, BL], F16, tag="hA")
                i_hnA = nc.vector.tensor_add(hnA, t1A, t2A)
                nc.sync.dma_start(out=outT[t][:, 0:HDT, :], in_=hnA)
                t1B = gp.tile([P, HDT, BL], F16, tag="t1B")
                i_t1B = nc.vector.tensor_mul(t1B, igB, ptB)
                t2B = gp.tile([P, HDT, BL], F16, tag="t2B")
                i_t2B = nc.vector.tensor_mul(t2B, fgB, qbB)
                hnB = hp.tile([P, HDT, BL], F16, tag="hB")
                i_hnB = nc.vector.tensor_add(hnB, t1B, t2B)
                nc.sync.dma_start(out=outT[t][:, HDT:DT, :], in_=hnB)

                _chain(
                    [i_qb0A, i_qbA, i_sA, i_dA, i_qbB, i_sB, i_dB,
                     i_t1A, i_t2A, i_hnA, i_t1B, i_t2B, i_hnB],
                    "dve-step-order",
                )
                _chain([i_igA, i_fgA, i_igB, i_fgB], "act-step-order")

                hA, hB = hnA, hnB


def build():
    if "nc" in _CACHE:
        return _CACHE["nc"]
    nc = bacc.Bacc("TRN2", target_bir_lowering=False, debug=False, num_devices=NCORES)
    xT = nc.dram_tensor("xT", [D, T, BL], F16, kind="ExternalInput").ap()
    w1 = nc.dram_tensor("w1", [P, DT, D], F16, kind="ExternalInput").ap()
    w2 = nc.dram_tensor("w2", [P, DT, D], F16, kind="ExternalInput").ap()
    b1 = nc.dram_tensor("b1", [P, DT], F32, kind="ExternalInput").ap()
    b2bc = nc.dram_tensor("b2bc", [P, DT, BL], F32, kind="ExternalInput").ap()
    h0 = nc.dram_tensor("h0", [P, DT, BL], F16, kind="ExternalInput").ap()
    outT = nc.dram_tensor("outT", [T, P, DT, BL], F16, kind="ExternalOutput").ap()
    with tile.TileContext(nc) as tc:
        nc._tc = tc
        _emit(nc, xT, w1, w2, b1, b2bc, h0, outT)
    nc.compile()
    _CACHE["nc"] = nc
    return nc


def make_in_maps(x, W1, b1, W2, b2, init_hx):
    x = np.asarray(x, dtype=np.float32)
    W1 = np.asarray(W1, dtype=np.float32)
    b1 = np.asarray(b1, dtype=np.float32)
    W2 = np.asarray(W2, dtype=np.float32)
    b2 = np.asarray(b2, dtype=np.float32)
    init_hx = np.asarray(init_hx, dtype=np.float32)

    w1s = np.ascontiguousarray(
        W1.T.reshape(DT, P, D).transpose(1, 0, 2)
    ).astype(np.float16)  # [din, dtile, e] = W1[e, d]
    w2s = np.ascontiguousarray(
        W2.T.reshape(DT, P, D).transpose(1, 0, 2)
    ).astype(np.float16)
    b1s = np.ascontiguousarray(b1.reshape(DT, P).T)  # [e_in, e_tile]
    b2bc = np.ascontiguousarray(
        np.broadcast_to(b2.reshape(DT, P).T[:, :, None], (P, DT, BL))
    )
    h0 = np.ascontiguousarray(
        np.broadcast_to(init_hx.reshape(DT, P).T[:, :, None], (P, DT, BL))
    ).astype(np.float16)

    in_maps = []
    for c in range(NCORES):
        xc = x[c * BL : (c + 1) * BL]  # [BL, T, D]
        xTc = np.ascontiguousarray(xc.transpose(2, 1, 0)).astype(np.float16)
        in_maps.append(
            {"xT": xTc, "w1": w1s, "w2": w2s, "b1": b1s, "b2bc": b2bc, "h0": h0}
        )
    return in_maps


def assemble(results):
    out = np.empty((B, T, D), dtype=np.float32)
    for c in range(NCORES):
        oT = results[c]["outT"]  # [T, P, DT, BL] f16
        out[c * BL : (c + 1) * BL] = (
            oT.transpose(3, 0, 2, 1).reshape(BL, T, D).astype(np.float32)
        )
    return out


def run(inputs, trace=False):
    nc = build()
    in_maps = make_in_maps(**inputs)
    res = run_bass_kernel_spmd(nc, in_maps, list(range(NCORES)), trace=trace)
    return assemble(res.results), res


def kernel(x, W1, b1, W2, b2, init_hx):
    out, _ = run(dict(x=x, W1=W1, b1=b1, W2=W2, b2=b2, init_hx=init_hx))
    return out


# revision 10
# speedup vs baseline: 1.4420x; 1.3078x over previous
"""ATR (twin-gate RNN) Trainium2 kernel.

  p = x @ W1.T + b1                       (batched GEMM over all T)
  h_t = sig(p_t+q_t)*p_t + sig(p_t-q_t)*q_t,  q_t = h_{t-1} @ W2.T + b2

Sharding: data-parallel over batch B=128 -> 16 per core across 8 NeuronCores,
zero cross-core communication. All tensors are kept on-device in a
[d-partition, batch-free] (transposed) layout; host prepares/unprepares.

Matmul operands are fp16 (measured ~5e-4 max rel err vs fp32 reference),
accumulation fp32 in PSUM.
"""

import contextlib

import numpy as np

import concourse.bass as bass
import concourse.mybir as mybir
import concourse.tile as tile
from concourse import bacc
from concourse.bass import ts
from concourse.bass_utils import run_bass_kernel_spmd
from concourse.tile import add_dep_helper


def _chain(insts, reason):
    """Force issue order within an engine queue (no extra semaphores).

    add_dep_helper(x, y) makes x wait on y, so the later op is the first arg.
    """
    for a, b in zip(insts, insts[1:]):
        add_dep_helper(b.ins, a.ins, sync=False, reason=reason)

B, T, D = 128, 256, 1024
NCORES = 8
BL = B // NCORES          # 16 batches per core
P = 128                   # partitions
DT = D // P               # 8 d-tiles
HDT = DT // 2             # 4 (half)
TW = 32                   # t-window per p-GEMM chunk (TW*BL = 512 cols)
NCH = T // TW             # 8 chunks
F16 = mybir.dt.float16
F32 = mybir.dt.float32

_CACHE = {}


def _emit(nc, xT, w1, w2, b1, b2bc, h0, outT):
    tc = nc._tc
    with contextlib.ExitStack() as ctx:
        singles = ctx.enter_context(tc.tile_pool(name="singles", bufs=1))
        w1_sb = singles.tile([P, DT, D], F16)
        nc.sync.dma_start(out=w1_sb, in_=w1)
        w2_sb = singles.tile([P, DT, D], F16)
        nc.sync.dma_start(out=w2_sb, in_=w2)
        b1_sb = singles.tile([P, DT], F32)
        nc.sync.dma_start(out=b1_sb, in_=b1)
        b2_sb = singles.tile([P, DT, BL], F32)
        nc.sync.dma_start(out=b2_sb, in_=b2bc)
        h0_sb = singles.tile([P, DT, BL], F16)
        nc.sync.dma_start(out=h0_sb, in_=h0)
        p_sb = singles.tile([P, DT, T, BL], F16)

        xT_r = xT.rearrange("(a p) t b -> a p t b", p=P)

        # ---- stage 1: p = x @ W1.T + b1, stored transposed as p_sb[e, t, b]
        with (
            tc.tile_pool(name="xin", bufs=3) as xpool,
            tc.tile_pool(name="pps", bufs=4, space="PSUM") as ppsum,
        ):
            for n in range(NCH):
                xn = xpool.tile([P, DT, TW, BL], F16)
                for dt in range(DT):
                    nc.sync.dma_start(out=xn[:, dt], in_=xT_r[dt, :, ts(n, TW), :])
                for e in range(DT):
                    ps = ppsum.tile([P, TW * BL], F32)
                    for dt in range(DT):
                        nc.tensor.matmul(
                            ps,
                            lhsT=w1_sb[:, dt, ts(e, P)],
                            rhs=xn[:, dt].rearrange("p t b -> p (t b)"),
                            start=(dt == 0),
                            stop=(dt == DT - 1),
                        )
                    nc.scalar.activation(
                        out=p_sb[:, e, ts(n, TW), :].rearrange("p t b -> p (t b)"),
                        in_=ps,
                        func=mybir.ActivationFunctionType.Identity,
                        bias=b1_sb[:, e : e + 1],
                        scale=1.0,
                    )

        # ---- stage 2: sequential recurrence over T
        with contextlib.ExitStack() as rctx:
            qps = rctx.enter_context(tc.tile_pool(name="qps", bufs=1, space="PSUM"))
            hp = rctx.enter_context(tc.tile_pool(name="hp", bufs=3))
            gp = rctx.enter_context(tc.tile_pool(name="gp", bufs=3))

            hA = h0_sb[:, 0:HDT, :]
            hB = h0_sb[:, HDT:DT, :]
            SIG = mybir.ActivationFunctionType.Sigmoid
            for t in range(T):
                # One PSUM bank per e-tile (free stride 512 f32 = one bank):
                # PSUM `start` clears has_written for its own bank only, so
                # each e-region can accumulate all 8 d-tiles in two separated
                # blocks (start on d0, stop on d7) while other banks' groups
                # interleave freely. Block order delays the hB-dependent
                # matmuls so the B gate chain of the previous step has time.
                qq = qps.tile([P, DT, 512], F32, tag="qq", name=f"qq_{t}")
                blocks = [
                    (0, HDT, 0),    # A x d0..3   (rhs hA)
                    (HDT, DT, 0),   # B x d0..3   (rhs hA)
                    (0, HDT, 1),    # A x d4..7   (rhs hB)
                    (HDT, DT, 1),   # B x d4..7   (rhs hB)
                ]
                for e0, e1, dh in blocks:
                    hsrc = hA if dh == 0 else hB
                    for e in range(e0, e1):
                        for dl in range(HDT):
                            d = dh * HDT + dl
                            nc.tensor.matmul(
                                qq[:, e, 0:BL],
                                lhsT=w2_sb[:, d, ts(e, P)],
                                rhs=hsrc[:, dl, :],
                                start=(d == 0),
                                stop=(d == DT - 1),
                            )
                qA = qq[:, 0:HDT, 0:BL]
                qB = qq[:, HDT:DT, 0:BL]

                # --- gates: A-half chain is the critical path into the next
                # step; B-half ops fill DVE idle slots while ACT runs the A
                # sigmoids.
                ptA = p_sb[:, 0:HDT, t, :]
                ptB = p_sb[:, HDT:DT, t, :]
                qbA = gp.tile([P, HDT, BL], F16, tag="qbA")
                i_qbA = nc.vector.tensor_add(qbA, qA, b2_sb[:, 0:HDT, :])
                sA = gp.tile([P, HDT, BL], F16, tag="sA")
                i_sA = nc.vector.tensor_add(sA, ptA, qbA)
                dA = gp.tile([P, HDT, BL], F16, tag="dA")
                i_dA = nc.vector.tensor_sub(dA, ptA, qbA)
                igA = gp.tile([P, HDT, BL], F16, tag="igA")
                i_igA = nc.scalar.activation(out=igA, in_=sA, func=SIG)
                fgA = gp.tile([P, HDT, BL], F16, tag="fgA")
                i_fgA = nc.scalar.activation(out=fgA, in_=dA, func=SIG)
                qbB = gp.tile([P, HDT, BL], F16, tag="qbB")
                i_qbB = nc.vector.tensor_add(qbB, qB, b2_sb[:, HDT:DT, :])
                sB = gp.tile([P, HDT, BL], F16, tag="sB")
                i_sB = nc.vector.tensor_add(sB, ptB, qbB)
                t1A = gp.tile([P, HDT, BL], F16, tag="t1A")
                i_t1A = nc.vector.tensor_mul(t1A, igA, ptA)
                t2A = gp.tile([P, HDT, BL], F16, tag="t2A")
                i_t2A = nc.vector.tensor_mul(t2A, fgA, qbA)
                hnA = hp.tile([P, HDT, BL], F16, tag="hA")
                i_hnA = nc.vector.tensor_add(hnA, t1A, t2A)
                nc.sync.dma_start(out=outT[t][:, 0:HDT, :], in_=hnA)
                dB = gp.tile([P, HDT, BL], F16, tag="dB")
                i_dB = nc.vector.tensor_sub(dB, ptB, qbB)
                igB = gp.tile([P, HDT, BL], F16, tag="igB")
                i_igB = nc.scalar.activation(out=igB, in_=sB, func=SIG)
                fgB = gp.tile([P, HDT, BL], F16, tag="fgB")
                i_fgB = nc.scalar.activation(out=fgB, in_=dB, func=SIG)
                t1B = gp.tile([P, HDT, BL], F16, tag="t1B")
                i_t1B = nc.vector.tensor_mul(t1B, igB, ptB)
                t2B = gp.tile([P, HDT, BL], F16, tag="t2B")
                i_t2B = nc.vector.tensor_mul(t2B, fgB, qbB)
                hnB = hp.tile([P, HDT, BL], F16, tag="hB")
                i_hnB = nc.vector.tensor_add(hnB, t1B, t2B)
                nc.sync.dma_start(out=outT[t][:, HDT:DT, :], in_=hnB)

                _chain(
                    [i_qbA, i_sA, i_dA, i_qbB, i_sB, i_t1A, i_t2A, i_hnA,
                     i_dB, i_t1B, i_t2B, i_hnB],
                    "dve-step-order",
                )
                _chain([i_igA, i_fgA, i_igB, i_fgB], "act-step-order")

                hA, hB = hnA, hnB


def build():
    if "nc" in _CACHE:
        return _CACHE["nc"]
    nc = bacc.Bacc("TRN2", target_bir_lowering=False, debug=False, num_devices=NCORES)
    xT = nc.dram_tensor("xT", [D, T, BL], F16, kind="ExternalInput").ap()
    w1 = nc.dram_tensor("w1", [P, DT, D], F16, kind="ExternalInput").ap()
    w2 = nc.dram_tensor("w2", [P, DT, D], F16, kind="ExternalInput").ap()
    b1 = nc.dram_tensor("b1", [P, DT], F32, kind="ExternalInput").ap()
    b2bc = nc.dram_tensor("b2bc", [P, DT, BL], F32, kind="ExternalInput").ap()
    h0 = nc.dram_tensor("h0", [P, DT, BL], F16, kind="ExternalInput").ap()
    outT = nc.dram_tensor("outT", [T, P, DT, BL], F16, kind="ExternalOutput").ap()
    with tile.TileContext(nc) as tc:
        nc._tc = tc
        _emit(nc, xT, w1, w2, b1, b2bc, h0, outT)
    nc.compile()
    _CACHE["nc"] = nc
    return nc


def make_in_maps(x, W1, b1, W2, b2, init_hx):
    x = np.asarray(x, dtype=np.float32)
    W1 = np.asarray(W1, dtype=np.float32)
    b1 = np.asarray(b1, dtype=np.float32)
    W2 = np.asarray(W2, dtype=np.float32)
    b2 = np.asarray(b2, dtype=np.float32)
    init_hx = np.asarray(init_hx, dtype=np.float32)

    w1s = np.ascontiguousarray(
        W1.T.reshape(DT, P, D).transpose(1, 0, 2)
    ).astype(np.float16)  # [din, dtile, e] = W1[e, d]
    w2s = np.ascontiguousarray(
        W2.T.reshape(DT, P, D).transpose(1, 0, 2)
    ).astype(np.float16)
    b1s = np.ascontiguousarray(b1.reshape(DT, P).T)  # [e_in, e_tile]
    b2bc = np.ascontiguousarray(
        np.broadcast_to(b2.reshape(DT, P).T[:, :, None], (P, DT, BL))
    )
    h0 = np.ascontiguousarray(
        np.broadcast_to(init_hx.reshape(DT, P).T[:, :, None], (P, DT, BL))
    ).astype(np.float16)

    in_maps = []
    for c in range(NCORES):
        xc = x[c * BL : (c + 1) * BL]  # [BL, T, D]
        xTc = np.ascontiguousarray(xc.transpose(2, 1, 0)).astype(np.float16)
        in_maps.append(
            {"xT": xTc, "w1": w1s, "w2": w2s, "b1": b1s, "b2bc": b2bc, "h0": h0}
        )
    return in_maps


def assemble(results):
    out = np.empty((B, T, D), dtype=np.float32)
    for c in range(NCORES):
        oT = results[c]["outT"]  # [T, P, DT, BL] f16
        out[c * BL : (c + 1) * BL] = (
            oT.transpose(3, 0, 2, 1).reshape(BL, T, D).astype(np.float32)
        )
    return out


def run(inputs, trace=False):
    nc = build()
    in_maps = make_in_maps(**inputs)
    res = run_bass_kernel_spmd(nc, in_maps, list(range(NCORES)), trace=trace)
    return assemble(res.results), res


def kernel(x, W1, b1, W2, b2, init_hx):
    out, _ = run(dict(x=x, W1=W1, b1=b1, W2=W2, b2=b2, init_hx=init_hx))
    return out


# revision 13
# speedup vs baseline: 1.4442x; 1.0015x over previous
"""ATR (twin-gate RNN) Trainium2 kernel.

  p = x @ W1.T + b1                       (batched GEMM over all T)
  h_t = sig(p_t+q_t)*p_t + sig(p_t-q_t)*q_t,  q_t = h_{t-1} @ W2.T + b2

Sharding: data-parallel over batch B=128 -> 16 per core across 8 NeuronCores,
zero cross-core communication. All tensors are kept on-device in a
[d-partition, batch-free] (transposed) layout; host prepares/unprepares.

Matmul operands are fp16 (measured ~5e-4 max rel err vs fp32 reference),
accumulation fp32 in PSUM.
"""

import contextlib

import numpy as np

import concourse.bass as bass
import concourse.mybir as mybir
import concourse.tile as tile
from concourse import bacc
from concourse.bass import ts
from concourse.bass_utils import run_bass_kernel_spmd
from concourse.tile import add_dep_helper


def _chain(insts, reason):
    """Force issue order within an engine queue (no extra semaphores).

    add_dep_helper(x, y) makes x wait on y, so the later op is the first arg.
    """
    for a, b in zip(insts, insts[1:]):
        add_dep_helper(b.ins, a.ins, sync=False, reason=reason)

B, T, D = 128, 256, 1024
NCORES = 8
BL = B // NCORES          # 16 batches per core
P = 128                   # partitions
DT = D // P               # 8 d-tiles
HDT = DT // 2             # 4 (half)
TW = 32                   # t-window per p-GEMM chunk (TW*BL = 512 cols)
NCH = T // TW             # 8 chunks
F16 = mybir.dt.float16
F32 = mybir.dt.float32

_CACHE = {}


def _emit(nc, xT, w1, w2, b1, b12, h0, outT):
    tc = nc._tc
    with contextlib.ExitStack() as ctx:
        singles = ctx.enter_context(tc.tile_pool(name="singles", bufs=1))
        w1_sb = singles.tile([P, DT, D], F16)
        nc.sync.dma_start(out=w1_sb, in_=w1)
        w2_sb = singles.tile([P, DT, D], F16)
        nc.sync.dma_start(out=w2_sb, in_=w2)
        b1_sb = singles.tile([P, DT], F32)
        nc.sync.dma_start(out=b1_sb, in_=b1)
        b12_sb = singles.tile([P, DT], F32)
        nc.sync.dma_start(out=b12_sb, in_=b12)
        h0_sb = singles.tile([P, DT, BL], F16)
        nc.sync.dma_start(out=h0_sb, in_=h0)
        p_sb = singles.tile([P, DT, T, BL], F16)
        pP_sb = singles.tile([P, DT, T, BL], F16)

        xT_r = xT.rearrange("(a p) t b -> a p t b", p=P)

        # ---- stage 1: p = x @ W1.T + b1, stored transposed as p_sb[e, t, b]
        with (
            tc.tile_pool(name="xin", bufs=3) as xpool,
            tc.tile_pool(name="pps", bufs=4, space="PSUM") as ppsum,
        ):
            for n in range(NCH):
                xn = xpool.tile([P, DT, TW, BL], F16)
                for dt in range(DT):
                    nc.sync.dma_start(out=xn[:, dt], in_=xT_r[dt, :, ts(n, TW), :])
                for e in range(DT):
                    ps = ppsum.tile([P, TW * BL], F32)
                    for dt in range(DT):
                        nc.tensor.matmul(
                            ps,
                            lhsT=w1_sb[:, dt, ts(e, P)],
                            rhs=xn[:, dt].rearrange("p t b -> p (t b)"),
                            start=(dt == 0),
                            stop=(dt == DT - 1),
                        )
                    nc.scalar.activation(
                        out=p_sb[:, e, ts(n, TW), :].rearrange("p t b -> p (t b)"),
                        in_=ps,
                        func=mybir.ActivationFunctionType.Identity,
                        bias=b1_sb[:, e : e + 1],
                        scale=1.0,
                    )
                    nc.vector.tensor_scalar_add(
                        pP_sb[:, e, ts(n, TW), :].rearrange("p t b -> p (t b)"),
                        ps,
                        b12_sb[:, e : e + 1],
                    )

        # ---- stage 2: sequential recurrence over T
        with contextlib.ExitStack() as rctx:
            qps = rctx.enter_context(tc.tile_pool(name="qps", bufs=1, space="PSUM"))
            hp = rctx.enter_context(tc.tile_pool(name="hp", bufs=3))
            gp = rctx.enter_context(tc.tile_pool(name="gp", bufs=3))

            hA = h0_sb[:, 0:HDT, :]
            hB = h0_sb[:, HDT:DT, :]
            SIG = mybir.ActivationFunctionType.Sigmoid
            for t in range(T):
                # One PSUM bank per e-tile (free stride 512 f32 = one bank):
                # PSUM `start` clears has_written for its own bank only, so
                # each e-region can accumulate all 8 d-tiles in two separated
                # blocks (start on d0, stop on d7) while other banks' groups
                # interleave freely. Block order delays the hB-dependent
                # matmuls so the B gate chain of the previous step has time.
                qq = qps.tile([P, DT, 512], F32, tag="qq", name=f"qq_{t}")
                blocks = [
                    (0, HDT, 0),        # A x d0..3   (rhs hA)
                    (HDT, DT - 1, 0),   # B(e4..6) x d0..3
                    (0, HDT, 1),        # A x d4..7   (rhs hB)
                    (DT - 1, DT, 0),    # B(e7) x d0..3
                    (HDT, DT, 1),       # B x d4..7   (rhs hB)
                ]
                for e0, e1, dh in blocks:
                    hsrc = hA if dh == 0 else hB
                    for e in range(e0, e1):
                        for dl in range(HDT):
                            d = dh * HDT + dl
                            nc.tensor.matmul(
                                qq[:, e, 0:BL],
                                lhsT=w2_sb[:, d, ts(e, P)],
                                rhs=hsrc[:, dl, :],
                                start=(d == 0),
                                stop=(d == DT - 1),
                            )
                qA = qq[:, 0:HDT, 0:BL]
                qB = qq[:, HDT:DT, 0:BL]

                # --- gates. Per half: s = pP + q (PSUM read), d = 2p - s,
                # qb = s - p (= q + b2), one sigmoid over [s; d], then
                # h = ig*p + fg*qb. The A chain feeds the next step's first
                # matmuls; B ops fill DVE slots while ACT runs the A sigmoid.
                ptA = p_sb[:, 0:HDT, t, :]
                ptB = p_sb[:, HDT:DT, t, :]
                pPA = pP_sb[:, 0:HDT, t, :]
                pPB = pP_sb[:, HDT:DT, t, :]
                M, SUB, ADD = (
                    mybir.AluOpType.mult,
                    mybir.AluOpType.subtract,
                    mybir.AluOpType.add,
                )

                sdA = gp.tile([P, 2, HDT, BL], F16, tag="sdA")
                i_sA = nc.vector.tensor_add(sdA[:, 0], pPA, qA)
                i_dA = nc.vector.scalar_tensor_tensor(
                    sdA[:, 1], ptA, 2.0, sdA[:, 0], M, SUB
                )
                qbA = gp.tile([P, HDT, BL], F16, tag="qbA")
                i_qbA = nc.vector.tensor_sub(qbA, sdA[:, 0], ptA)
                igfgA = gp.tile([P, 2, HDT, BL], F16, tag="igfgA")
                i_igA = nc.scalar.activation(
                    out=igfgA.rearrange("p s d b -> p (s d b)"),
                    in_=sdA.rearrange("p s d b -> p (s d b)"),
                    func=SIG,
                )
                sdB = gp.tile([P, 2, HDT, BL], F16, tag="sdB")
                i_sB = nc.vector.tensor_add(sdB[:, 0], pPB, qB)
                i_dB = nc.vector.scalar_tensor_tensor(
                    sdB[:, 1], ptB, 2.0, sdB[:, 0], M, SUB
                )
                qbB = gp.tile([P, HDT, BL], F16, tag="qbB")
                i_qbB = nc.vector.tensor_sub(qbB, sdB[:, 0], ptB)
                igfgB = gp.tile([P, 2, HDT, BL], F16, tag="igfgB")
                i_igB = nc.scalar.activation(
                    out=igfgB.rearrange("p s d b -> p (s d b)"),
                    in_=sdB.rearrange("p s d b -> p (s d b)"),
                    func=SIG,
                )
                t1A = gp.tile([P, HDT, BL], F16, tag="t1A")
                i_t1A = nc.vector.tensor_mul(t1A, igfgA[:, 0], ptA)
                t2A = gp.tile([P, HDT, BL], F16, tag="t2A")
                i_t2A = nc.vector.tensor_mul(t2A, igfgA[:, 1], qbA)
                hnA = hp.tile([P, HDT, BL], F16, tag="hA")
                i_hnA = nc.vector.tensor_add(hnA, t1A, t2A)
                nc.sync.dma_start(out=outT[t][:, 0:HDT, :], in_=hnA)
                t1B = gp.tile([P, HDT, BL], F16, tag="t1B")
                i_t1B = nc.vector.tensor_mul(t1B, igfgB[:, 0], ptB)
                t2B = gp.tile([P, HDT, BL], F16, tag="t2B")
                i_t2B = nc.vector.tensor_mul(t2B, igfgB[:, 1], qbB)
                hnB = hp.tile([P, HDT, BL], F16, tag="hB")
                i_hnB = nc.vector.tensor_add(hnB, t1B, t2B)
                nc.sync.dma_start(out=outT[t][:, HDT:DT, :], in_=hnB)

                _chain(
                    [i_sA, i_dA, i_qbA, i_sB, i_dB, i_qbB,
                     i_t1A, i_t2A, i_hnA, i_t1B, i_t2B, i_hnB],
                    "dve-step-order",
                )
                _chain([i_igA, i_igB], "act-step-order")

                hA, hB = hnA, hnB


def build():
    if "nc" in _CACHE:
        return _CACHE["nc"]
    nc = bacc.Bacc("TRN2", target_bir_lowering=False, debug=False, num_devices=NCORES)
    xT = nc.dram_tensor("xT", [D, T, BL], F16, kind="ExternalInput").ap()
    w1 = nc.dram_tensor("w1", [P, DT, D], F16, kind="ExternalInput").ap()
    w2 = nc.dram_tensor("w2", [P, DT, D], F16, kind="ExternalInput").ap()
    b1 = nc.dram_tensor("b1", [P, DT], F32, kind="ExternalInput").ap()
    b12 = nc.dram_tensor("b12", [P, DT], F32, kind="ExternalInput").ap()
    h0 = nc.dram_tensor("h0", [P, DT, BL], F16, kind="ExternalInput").ap()
    outT = nc.dram_tensor("outT", [T, P, DT, BL], F16, kind="ExternalOutput").ap()
    with tile.TileContext(nc) as tc:
        nc._tc = tc
        _emit(nc, xT, w1, w2, b1, b12, h0, outT)
    nc.compile()
    _CACHE["nc"] = nc
    return nc


def make_in_maps(x, W1, b1, W2, b2, init_hx):
    x = np.asarray(x, dtype=np.float32)
    W1 = np.asarray(W1, dtype=np.float32)
    b1 = np.asarray(b1, dtype=np.float32)
    W2 = np.asarray(W2, dtype=np.float32)
    b2 = np.asarray(b2, dtype=np.float32)
    init_hx = np.asarray(init_hx, dtype=np.float32)

    w1s = np.ascontiguousarray(
        W1.T.reshape(DT, P, D).transpose(1, 0, 2)
    ).astype(np.float16)  # [din, dtile, e] = W1[e, d]
    w2s = np.ascontiguousarray(
        W2.T.reshape(DT, P, D).transpose(1, 0, 2)
    ).astype(np.float16)
    b1s = np.ascontiguousarray(b1.reshape(DT, P).T)  # [e_in, e_tile]
    b12s = np.ascontiguousarray((b1 + b2).reshape(DT, P).T)
    h0 = np.ascontiguousarray(
        np.broadcast_to(init_hx.reshape(DT, P).T[:, :, None], (P, DT, BL))
    ).astype(np.float16)

    in_maps = []
    for c in range(NCORES):
        xc = x[c * BL : (c + 1) * BL]  # [BL, T, D]
        xTc = np.ascontiguousarray(xc.transpose(2, 1, 0)).astype(np.float16)
        in_maps.append(
            {"xT": xTc, "w1": w1s, "w2": w2s, "b1": b1s, "b12": b12s, "h0": h0}
        )
    return in_maps


def assemble(results):
    out = np.empty((B, T, D), dtype=np.float32)
    for c in range(NCORES):
        oT = results[c]["outT"]  # [T, P, DT, BL] f16
        out[c * BL : (c + 1) * BL] = (
            oT.transpose(3, 0, 2, 1).reshape(BL, T, D).astype(np.float32)
        )
    return out


def run(inputs, trace=False):
    nc = build()
    in_maps = make_in_maps(**inputs)
    res = run_bass_kernel_spmd(nc, in_maps, list(range(NCORES)), trace=trace)
    return assemble(res.results), res


def kernel(x, W1, b1, W2, b2, init_hx):
    out, _ = run(dict(x=x, W1=W1, b1=b1, W2=W2, b2=b2, init_hx=init_hx))
    return out


# revision 14
# speedup vs baseline: 1.4489x; 1.0033x over previous
"""ATR (twin-gate RNN) Trainium2 kernel.

  p = x @ W1.T + b1                       (batched GEMM over all T)
  h_t = sig(p_t+q_t)*p_t + sig(p_t-q_t)*q_t,  q_t = h_{t-1} @ W2.T + b2

Sharding: data-parallel over batch B=128 -> 16 per core across 8 NeuronCores,
zero cross-core communication. All tensors are kept on-device in a
[d-partition, batch-free] (transposed) layout; host prepares/unprepares.

Matmul operands are fp16 (measured ~5e-4 max rel err vs fp32 reference),
accumulation fp32 in PSUM.
"""

import contextlib

import numpy as np

import concourse.bass as bass
import concourse.mybir as mybir
import concourse.tile as tile
from concourse import bacc
from concourse.bass import ts
from concourse.bass_utils import run_bass_kernel_spmd
from concourse.tile import add_dep_helper


def _chain(insts, reason):
    """Force issue order within an engine queue (no extra semaphores).

    add_dep_helper(x, y) makes x wait on y, so the later op is the first arg.
    """
    for a, b in zip(insts, insts[1:]):
        add_dep_helper(b.ins, a.ins, sync=False, reason=reason)

B, T, D = 128, 256, 1024
NCORES = 8
BL = B // NCORES          # 16 batches per core
P = 128                   # partitions
DT = D // P               # 8 d-tiles
HDT = DT // 2             # 4 (half)
TW = 32                   # t-window per p-GEMM chunk (TW*BL = 512 cols)
NCH = T // TW             # 8 chunks
F16 = mybir.dt.float16
F32 = mybir.dt.float32

_CACHE = {}


def _emit(nc, xT, w1, w2, b1, b12, h0, outT):
    tc = nc._tc
    with contextlib.ExitStack() as ctx:
        singles = ctx.enter_context(tc.tile_pool(name="singles", bufs=1))
        w1_sb = singles.tile([P, DT, D], F16)
        nc.sync.dma_start(out=w1_sb, in_=w1)
        w2_sb = singles.tile([P, DT, D], F16)
        nc.sync.dma_start(out=w2_sb, in_=w2)
        b1_sb = singles.tile([P, DT], F32)
        nc.sync.dma_start(out=b1_sb, in_=b1)
        b12_sb = singles.tile([P, DT], F32)
        nc.sync.dma_start(out=b12_sb, in_=b12)
        h0_sb = singles.tile([P, DT, BL], F16)
        nc.sync.dma_start(out=h0_sb, in_=h0)
        p_sb = singles.tile([P, DT, T, BL], F16)
        pP_sb = singles.tile([P, DT, T, BL], F16)

        xT_r = xT.rearrange("(a p) t b -> a p t b", p=P)

        # ---- stage 1: p = x @ W1.T + b1, stored transposed as p_sb[e, t, b]
        with (
            tc.tile_pool(name="xin", bufs=3) as xpool,
            tc.tile_pool(name="pps", bufs=4, space="PSUM") as ppsum,
        ):
            for n in range(NCH):
                xn = xpool.tile([P, DT, TW, BL], F16)
                for dt in range(DT):
                    nc.sync.dma_start(out=xn[:, dt], in_=xT_r[dt, :, ts(n, TW), :])
                for e in range(DT):
                    ps = ppsum.tile([P, TW * BL], F32)
                    for dt in range(DT):
                        nc.tensor.matmul(
                            ps,
                            lhsT=w1_sb[:, dt, ts(e, P)],
                            rhs=xn[:, dt].rearrange("p t b -> p (t b)"),
                            start=(dt == 0),
                            stop=(dt == DT - 1),
                        )
                    nc.scalar.activation(
                        out=p_sb[:, e, ts(n, TW), :].rearrange("p t b -> p (t b)"),
                        in_=ps,
                        func=mybir.ActivationFunctionType.Identity,
                        bias=b1_sb[:, e : e + 1],
                        scale=1.0,
                    )
                    nc.vector.tensor_scalar_add(
                        pP_sb[:, e, ts(n, TW), :].rearrange("p t b -> p (t b)"),
                        ps,
                        b12_sb[:, e : e + 1],
                    )

        # ---- stage 2: sequential recurrence over T
        with contextlib.ExitStack() as rctx:
            qps = rctx.enter_context(tc.tile_pool(name="qps", bufs=1, space="PSUM"))
            hp = rctx.enter_context(tc.tile_pool(name="hp", bufs=3))
            gp = rctx.enter_context(tc.tile_pool(name="gp", bufs=3))

            hA = h0_sb[:, 0:HDT, :]
            hB = h0_sb[:, HDT:DT, :]
            SIG = mybir.ActivationFunctionType.Sigmoid
            for t in range(T):
                # One PSUM bank per e-tile (free stride 512 f32 = one bank):
                # PSUM `start` clears has_written for its own bank only, so
                # each e-region can accumulate all 8 d-tiles in two separated
                # blocks (start on d0, stop on d7) while other banks' groups
                # interleave freely. Block order delays the hB-dependent
                # matmuls so the B gate chain of the previous step has time.
                qqA = qps.tile([P, HDT, 512], F32, tag="qqA", name=f"qqA_{t}")
                qqB = qps.tile([P, HDT, 512], F32, tag="qqB", name=f"qqB_{t}")
                blocks = [
                    (qqA, 0, 0),   # A x d0..3   (rhs hA)
                    (qqB, HDT, 0),  # B x d0..3   (rhs hA)
                    (qqA, 0, 1),   # A x d4..7   (rhs hB)
                    (qqB, HDT, 1),  # B x d4..7   (rhs hB)
                ]
                for qt_, eoff, dh in blocks:
                    hsrc = hA if dh == 0 else hB
                    for el in range(HDT):
                        for dl in range(HDT):
                            d = dh * HDT + dl
                            nc.tensor.matmul(
                                qt_[:, el, 0:BL],
                                lhsT=w2_sb[:, d, ts(eoff + el, P)],
                                rhs=hsrc[:, dl, :],
                                start=(d == 0),
                                stop=(d == DT - 1),
                            )
                qA = qqA[:, :, 0:BL]
                qB = qqB[:, :, 0:BL]

                # --- gates. Per half: s = pP + q (PSUM read), d = 2p - s,
                # qb = s - p (= q + b2), one sigmoid over [s; d], then
                # h = ig*p + fg*qb. The A chain feeds the next step's first
                # matmuls; B ops fill DVE slots while ACT runs the A sigmoid.
                ptA = p_sb[:, 0:HDT, t, :]
                ptB = p_sb[:, HDT:DT, t, :]
                pPA = pP_sb[:, 0:HDT, t, :]
                pPB = pP_sb[:, HDT:DT, t, :]
                M, SUB, ADD = (
                    mybir.AluOpType.mult,
                    mybir.AluOpType.subtract,
                    mybir.AluOpType.add,
                )

                sdA = gp.tile([P, 2, HDT, BL], F16, tag="sdA")
                i_sA = nc.vector.tensor_add(sdA[:, 0], pPA, qA)
                i_dA = nc.vector.scalar_tensor_tensor(
                    sdA[:, 1], ptA, 2.0, sdA[:, 0], M, SUB
                )
                qbA = gp.tile([P, HDT, BL], F16, tag="qbA")
                i_qbA = nc.vector.tensor_sub(qbA, sdA[:, 0], ptA)
                igfgA = gp.tile([P, 2, HDT, BL], F16, tag="igfgA")
                i_igA = nc.scalar.activation(
                    out=igfgA.rearrange("p s d b -> p (s d b)"),
                    in_=sdA.rearrange("p s d b -> p (s d b)"),
                    func=SIG,
                )
                sdB = gp.tile([P, 2, HDT, BL], F16, tag="sdB")
                i_sB = nc.vector.tensor_add(sdB[:, 0], pPB, qB)
                i_dB = nc.vector.scalar_tensor_tensor(
                    sdB[:, 1], ptB, 2.0, sdB[:, 0], M, SUB
                )
                qbB = gp.tile([P, HDT, BL], F16, tag="qbB")
                i_qbB = nc.vector.tensor_sub(qbB, sdB[:, 0], ptB)
                igfgB = gp.tile([P, 2, HDT, BL], F16, tag="igfgB")
                i_igB = nc.scalar.activation(
                    out=igfgB.rearrange("p s d b -> p (s d b)"),
                    in_=sdB.rearrange("p s d b -> p (s d b)"),
                    func=SIG,
                )
                t1A = gp.tile([P, HDT, BL], F16, tag="t1A")
                i_t1A = nc.vector.tensor_mul(t1A, igfgA[:, 0], ptA)
                t2A = gp.tile([P, HDT, BL], F16, tag="t2A")
                i_t2A = nc.vector.tensor_mul(t2A, igfgA[:, 1], qbA)
                hnA = hp.tile([P, HDT, BL], F16, tag="hA")
                i_hnA = nc.vector.tensor_add(hnA, t1A, t2A)
                nc.sync.dma_start(out=outT[t][:, 0:HDT, :], in_=hnA)
                t1B = gp.tile([P, HDT, BL], F16, tag="t1B")
                i_t1B = nc.vector.tensor_mul(t1B, igfgB[:, 0], ptB)
                t2B = gp.tile([P, HDT, BL], F16, tag="t2B")
                i_t2B = nc.vector.tensor_mul(t2B, igfgB[:, 1], qbB)
                hnB = hp.tile([P, HDT, BL], F16, tag="hB")
                i_hnB = nc.vector.tensor_add(hnB, t1B, t2B)
                nc.sync.dma_start(out=outT[t][:, HDT:DT, :], in_=hnB)

                _chain(
                    [i_sA, i_dA, i_qbA, i_sB, i_dB, i_qbB,
                     i_t1A, i_t2A, i_hnA, i_t1B, i_t2B, i_hnB],
                    "dve-step-order",
                )
                _chain([i_igA, i_igB], "act-step-order")

                hA, hB = hnA, hnB


def build():
    if "nc" in _CACHE:
        return _CACHE["nc"]
    nc = bacc.Bacc("TRN2", target_bir_lowering=False, debug=False, num_devices=NCORES)
    xT = nc.dram_tensor("xT", [D, T, BL], F16, kind="ExternalInput").ap()
    w1 = nc.dram_tensor("w1", [P, DT, D], F16, kind="ExternalInput").ap()
    w2 = nc.dram_tensor("w2", [P, DT, D], F16, kind="ExternalInput").ap()
    b1 = nc.dram_tensor("b1", [P, DT], F32, kind="ExternalInput").ap()
    b12 = nc.dram_tensor("b12", [P, DT], F32, kind="ExternalInput").ap()
    h0 = nc.dram_tensor("h0", [P, DT, BL], F16, kind="ExternalInput").ap()
    outT = nc.dram_tensor("outT", [T, P, DT, BL], F16, kind="ExternalOutput").ap()
    with tile.TileContext(nc) as tc:
        nc._tc = tc
        _emit(nc, xT, w1, w2, b1, b12, h0, outT)
    nc.compile()
    _CACHE["nc"] = nc
    return nc


def make_in_maps(x, W1, b1, W2, b2, init_hx):
    x = np.asarray(x, dtype=np.float32)
    W1 = np.asarray(W1, dtype=np.float32)
    b1 = np.asarray(b1, dtype=np.float32)
    W2 = np.asarray(W2, dtype=np.float32)
    b2 = np.asarray(b2, dtype=np.float32)
    init_hx = np.asarray(init_hx, dtype=np.float32)

    w1s = np.ascontiguousarray(
        W1.T.reshape(DT, P, D).transpose(1, 0, 2)
    ).astype(np.float16)  # [din, dtile, e] = W1[e, d]
    w2s = np.ascontiguousarray(
        W2.T.reshape(DT, P, D).transpose(1, 0, 2)
    ).astype(np.float16)
    b1s = np.ascontiguousarray(b1.reshape(DT, P).T)  # [e_in, e_tile]
    b12s = np.ascontiguousarray((b1 + b2).reshape(DT, P).T)
    h0 = np.ascontiguousarray(
        np.broadcast_to(init_hx.reshape(DT, P).T[:, :, None], (P, DT, BL))
    ).astype(np.float16)

    in_maps = []
    for c in range(NCORES):
        xc = x[c * BL : (c + 1) * BL]  # [BL, T, D]
        xTc = np.ascontiguousarray(xc.transpose(2, 1, 0)).astype(np.float16)
        in_maps.append(
            {"xT": xTc, "w1": w1s, "w2": w2s, "b1": b1s, "b12": b12s, "h0": h0}
        )
    return in_maps


def assemble(results):
    out = np.empty((B, T, D), dtype=np.float32)
    for c in range(NCORES):
        oT = results[c]["outT"]  # [T, P, DT, BL] f16
        out[c * BL : (c + 1) * BL] = (
            oT.transpose(3, 0, 2, 1).reshape(BL, T, D).astype(np.float32)
        )
    return out


def run(inputs, trace=False):
    nc = build()
    in_maps = make_in_maps(**inputs)
    res = run_bass_kernel_spmd(nc, in_maps, list(range(NCORES)), trace=trace)
    return assemble(res.results), res


def kernel(x, W1, b1, W2, b2, init_hx):
    out, _ = run(dict(x=x, W1=W1, b1=b1, W2=W2, b2=b2, init_hx=init_hx))
    return out
